# revision 35
# baseline (speedup 1.0000x reference)
"""BiGRU encoder kernel for 8 Trainium2 NeuronCores.

Strategy (v2 — mixed precision bf16/fp8):
  - Masked GRUs over fixed position ranges (see v1): forward runs positions
    ascending into the center, backward descending; a sample of length l only
    starts updating z-steps from its start; pre-start samples are forced to
    z==1 (h'==h==0 exactly) by adding +SCALE*BIG to the z-gate preactivation.
  - Sort samples by window_len, deal round-robin to 8 cores; per core two
    batch tiles of 512 sorted samples; per GRU step only the suffix of
    samples that needs the step is processed (width gran 32).
  - Precision: z,n input projections in bf16 (accuracy-critical), r input
    projection and ALL hidden projections in fp8e4m3 with DoubleRow matmuls
    (2 k-chunks per instruction) — GRU recurrence is contractive, so fp8
    noise on the hidden path washes out (measured 1.0e-2 vs 2e-2 budget).
    All weights pre-scaled x16 host-side (fp8 subnormal floor), undone by
    the activation `scale=1/16`.
  - z-mask applied as a rank-1 matmul (stationary [1,128]=640, moving 0/1
    row) accumulated into the z PSUM group — no DVE mask add.
  - n-gate: tt=(ghn+16*bhh_n)*r on DVE (bf16 out), then an identity matmul
    accumulates tt into the xn PSUM group — no separate ss add.
  - h carried in bf16; cast to fp8 (scalar engine) for the next hidden
    matmul.  h-update chain (dd,e,ho) runs merged over 2-kchunk halves in
    bf16 (DVE 2x mode), halves staggered so the next step's first DoubleRow
    pair can start after half 0.
  - The two batch tiles are interleaved step-by-step, and each step is
    emitted in two phases (A: DMAs+projection matmuls+r/z ACTs+tt, B:
    id-matmul+n ACT+h update+cast) so the PE always has the other tile's
    matmul stream to chew while one tile's gate chain drains.
"""

import os
from contextlib import ExitStack

import numpy as np
import ml_dtypes

import concourse.bacc as bacc
import concourse.tile as tile
from concourse import mybir
from concourse.bass_utils import run_bass_kernel_spmd
from concourse.masks import make_identity

NCORES = 8
B, T, D, H = 8192, 15, 512, 512
G = 3 * H
BIG = 40.0
SCALE = 16.0
S = 512  # samples per batch tile
F32 = mybir.dt.float32
BF16 = mybir.dt.bfloat16
F8 = mybir.dt.float8e4
DR_MIN = int(os.environ.get("GRU_DRMIN", "320"))  # min width for DoubleRow

ACT = mybir.ActivationFunctionType
ALU = mybir.AluOpType
DRM = mybir.MatmulPerfMode.DoubleRow

_PROGRAM_CACHE = {}
LAST_RESULT = None


def _build_program(sched):
    """sched: per tile, (f_steps, b_steps); each step = (width, masked)."""
    ntiles = len(sched)
    Bc = S * ntiles
    nc = bacc.Bacc("TRN2", target_bir_lowering=False, debug=False,
                   num_devices=NCORES)

    xb_d = nc.dram_tensor("xb", [T, D, Bc], BF16, kind="ExternalInput")
    x8_d = nc.dram_tensor("x8", [T, D, Bc], F8, kind="ExternalInput")
    wzn_d = {'f': nc.dram_tensor("wnf", [D, H], BF16, kind="ExternalInput"),
             'b': nc.dram_tensor("wnb", [D, H], BF16, kind="ExternalInput")}
    wr8_d = {'f': nc.dram_tensor("wrz8f", [D, 2 * H], F8, kind="ExternalInput"),
             'b': nc.dram_tensor("wrz8b", [D, 2 * H], F8, kind="ExternalInput")}
    wh8_d = {'f': nc.dram_tensor("wh8f", [H, G], F8, kind="ExternalInput"),
             'b': nc.dram_tensor("wh8b", [H, G], F8, kind="ExternalInput")}
    w1_d = nc.dram_tensor("w1", [2 * H, H], BF16, kind="ExternalInput")
    w2_d = nc.dram_tensor("w2", [H, H], BF16, kind="ExternalInput")
    bias_d = nc.dram_tensor("bias", [40, 128], F32, kind="ExternalInput")
    m_d = {'f': nc.dram_tensor("maskzf", [8, Bc], BF16, kind="ExternalInput"),
           'b': nc.dram_tensor("maskzb", [8, Bc], BF16, kind="ExternalInput")}
    y_d = nc.dram_tensor("y", [Bc, H], F32, kind="ExternalOutput")

    with tile.TileContext(nc) as tc, ExitStack() as ctx:
        const = ctx.enter_context(tc.tile_pool(name="const", bufs=1))
        xpool = ctx.enter_context(tc.tile_pool(name="x", bufs=4))
        hpool = ctx.enter_context(tc.tile_pool(name="h", bufs=6))
        h8pool = ctx.enter_context(tc.tile_pool(name="h8", bufs=4))
        gpool = ctx.enter_context(tc.tile_pool(name="g", bufs=2))
        opool = ctx.enter_context(tc.tile_pool(name="o", bufs=4))
        rzps = ctx.enter_context(tc.tile_pool(name="rz", bufs=4, space="PSUM"))
        xpps = ctx.enter_context(tc.tile_pool(name="xp", bufs=2, space="PSUM"))
        ghps = ctx.enter_context(tc.tile_pool(name="gh", bufs=2, space="PSUM"))

        def load_w(dram, kchunks, cols, dt, name, eng):
            t_ = const.tile([128, kchunks, cols], dt, name=name)
            src = dram.rearrange("(c k) g -> k c g", k=128)
            for c in range(kchunks):
                eng.dma_start(t_[:, c, :], src[:, c, :])
            return t_

        # per-kchunk DMAs interleaved across both directions and two queues
        # so every chain's first-needed weight chunks land within ~3us
        wzn, wr8, wh8 = {}, {}, {}
        for d in 'fb':
            wzn[d] = const.tile([128, 4, H], BF16, name=f"wn{d}")
            wr8[d] = const.tile([128, 4, 2 * H], F8, name=f"wrz8{d}")
            wh8[d] = const.tile([128, 4, G], F8, name=f"wh8{d}")
        for c in range(4):
            for d, eng in (('f', nc.scalar), ('b', nc.scalar)):
                for t_, dram in ((wzn[d], wzn_d[d]), (wr8[d], wr8_d[d]),
                                 (wh8[d], wh8_d[d])):
                    src_ = dram.rearrange("(c k) g -> k c g", k=128)
                    eng.dma_start(t_[:, c, :], src_[:, c, :])
        w1 = load_w(w1_d, 8, H, BF16, "w1", nc.gpsimd)
        w2 = load_w(w2_d, 4, H, BF16, "w2", nc.gpsimd)
        bt = const.tile([128, 40], F32)
        nc.gpsimd.dma_start(bt[:], bias_d.rearrange("n p -> p n"))
        ident = const.tile([128, 128], BF16)
        make_identity(nc, ident[:])

        class St:  # per (dir, tile) recurrence state
            h = None
            h8 = None
            so = None

        def phase_a(d, t, steps, j, st):
            """DMAs, projection matmuls, mask, r/z ACTs, tt."""
            w, masked = steps[j]
            nsteps = len(steps)
            first = j == 0
            so = S - w
            s0 = t * S
            a0 = s0 + so
            bb = 0 if d == 'f' else 16
            pos = (8 - nsteps + j) if d == 'f' else (6 + nsteps - j)
            use_dr = w >= DR_MIN

            mt = None
            if masked:
                mt = gpool.tile([128, S], BF16, tag="mt", bufs=2, name="mt")
                nc.gpsimd.dma_start(
                    mt[:, :w],
                    m_d[d][8 - nsteps + j, a0:s0 + S].partition_broadcast(128))
            xt = xpool.tile([128, 4, S], BF16, tag="x", name="xt")
            nc.sync.dma_start(
                xt[:, :, :w],
                xb_d[pos].rearrange("(c k) s -> k c s", k=128)[:, :, a0:s0 + S])
            x8 = xpool.tile([128, 4, S], F8, tag="x8", bufs=4, name="x8t")
            nc.sync.dma_start(
                x8[:, :, :w],
                x8_d[pos].rearrange("(c k) s -> k c s", k=128)[:, :, a0:s0 + S])

            rps, zps, xpn, ghn = [], [], [], []
            for i in range(4):
                rps.append(rzps.tile([128, 512], F32, tag="rz", name=f"rps{i}"))
                zps.append(rzps.tile([128, 512], F32, tag="rz", name=f"zps{i}"))
                xpn.append(xpps.tile([128, 512], F32, tag="xp", name=f"xpn{i}"))
                if not first:
                    ghn.append(ghps.tile([128, 512], F32, tag="gh",
                                         name=f"ghn{i}"))
            wn, wh, wrz = wzn[d], wh8[d], wr8[d]
            for i in range(4):
                c0, c1 = i * 128, (i + 1) * 128
                # n input projection (bf16); group closes here, ss is on DVE
                for k in range(4):
                    nc.tensor.matmul(xpn[i][:, :w], wn[:, k, c0:c1],
                                     xt[:, k, :w], start=k == 0, stop=k == 3)
                # r and z input projections (fp8)
                if use_dr:
                    for kp in (0, 2):
                        nc.tensor.matmul(rps[i][:, :w], wrz[:, kp:kp + 2, c0:c1],
                                         x8[:, kp:kp + 2, :w], start=kp == 0,
                                         stop=first and kp == 2, perf_mode=DRM)
                        nc.tensor.matmul(zps[i][:, :w],
                                         wrz[:, kp:kp + 2, H + c0:H + c1],
                                         x8[:, kp:kp + 2, :w], start=kp == 0,
                                         stop=first and kp == 2,
                                         perf_mode=DRM)
                else:
                    for k in range(4):
                        nc.tensor.matmul(rps[i][:, :w], wrz[:, k, c0:c1],
                                         x8[:, k, :w], start=k == 0,
                                         stop=first and k == 3)
                        nc.tensor.matmul(zps[i][:, :w], wrz[:, k, H + c0:H + c1],
                                         x8[:, k, :w], start=k == 0,
                                         stop=first and k == 3)
                # hidden projections (fp8), accumulate into same groups
                if not first:
                    h8 = st.h8
                    if use_dr:
                        for kp in (0, 2):
                            hk = h8[:, kp:kp + 2, so:]
                            last = kp == 2
                            nc.tensor.matmul(rps[i][:, :w], wh[:, kp:kp + 2, c0:c1],
                                             hk, start=False, stop=last,
                                             perf_mode=DRM)
                            nc.tensor.matmul(zps[i][:, :w],
                                             wh[:, kp:kp + 2, H + c0:H + c1],
                                             hk, start=False, stop=last,
                                             perf_mode=DRM)
                            nc.tensor.matmul(ghn[i][:, :w],
                                             wh[:, kp:kp + 2, 2 * H + c0:2 * H + c1],
                                             hk, start=kp == 0, stop=last,
                                             perf_mode=DRM)
                    else:
                        for k in range(4):
                            hk = st.h8[:, k, so:]
                            last = k == 3
                            nc.tensor.matmul(rps[i][:, :w], wh[:, k, c0:c1],
                                             hk, start=False, stop=last)
                            nc.tensor.matmul(zps[i][:, :w], wh[:, k, H + c0:H + c1],
                                             hk, start=False, stop=last)
                            nc.tensor.matmul(ghn[i][:, :w],
                                             wh[:, k, 2 * H + c0:2 * H + c1],
                                             hk, start=k == 0, stop=last)
            r_g = gpool.tile([128, 4, S], BF16, tag="r", bufs=2, name="r_g")
            tt = gpool.tile([128, 4, S], BF16, tag="tt", bufs=2, name="tt")
            z_g, n_g = st.z_g, st.n_g
            for i in range(4):
                nc.scalar.activation(r_g[:, i, :w], rps[i][:, :w], ACT.Sigmoid,
                                     bias=bt[:, bb + i:bb + i + 1],
                                     scale=1.0 / SCALE)
                if masked:
                    zr = gpool.tile([128, 512], BF16, tag="zr", bufs=2,
                                    name="zr")
                    nc.scalar.activation(zr[:, :w], zps[i][:, :w], ACT.Sigmoid,
                                         bias=bt[:, bb + 4 + i:bb + 5 + i],
                                         scale=1.0 / SCALE)
                    nc.vector.tensor_max(z_g[:, i, :w], zr[:, :w], mt[:, :w])
                else:
                    nc.scalar.activation(z_g[:, i, :w], zps[i][:, :w],
                                         ACT.Sigmoid,
                                         bias=bt[:, bb + 4 + i:bb + 5 + i],
                                         scale=1.0 / SCALE)
                if first:
                    nc.vector.tensor_scalar_mul(tt[:, i, :w], r_g[:, i, :w],
                                                bt[:, bb + 8 + i:bb + 9 + i])
                else:
                    nc.vector.scalar_tensor_tensor(
                        tt[:, i, :w], ghn[i][:, :w],
                        bt[:, bb + 8 + i:bb + 9 + i], r_g[:, i, :w],
                        op0=ALU.add, op1=ALU.mult)
                # ss and the n ACT stay in phase A so the xpn bank and the
                # ss buffer are both freed within the phase (no cross-phase
                # PSUM/buffer WAR edges -> no scheduling deadlock)
                ss = gpool.tile([128, 512], F32, tag="ss", bufs=4, name="ss")
                nc.vector.tensor_add(ss[:, :w], tt[:, i, :w], xpn[i][:, :w])
                nc.scalar.activation(n_g[:, i, :w], ss[:, :w], ACT.Tanh,
                                     bias=bt[:, bb + 12 + i:bb + 13 + i],
                                     scale=1.0 / SCALE)
            return w, so

        def phase_b(d, t, steps, j, st, pa):
            """h update, fp8 cast."""
            w, so = pa
            z_g, n_g = st.z_g, st.n_g
            nsteps = len(steps)
            first = j == 0
            last = j == nsteps - 1
            h_next = hpool.tile([128, 4, S], BF16, tag="hf" if last else "h",
                                bufs=4 if last else 6, name="h")
            if not last:
                nw = steps[j + 1][0]
                nso = S - nw
                if nso < so:
                    nc.gpsimd.memset(h_next[:, :, nso:so], 0.0)
            e = gpool.tile([128, 4, S], BF16, tag="e", bufs=2, name="e")
            dd = None
            if not first:
                dd = gpool.tile([128, 4, S], BF16, tag="dd", bufs=2, name="dd")
            for hh in (0, 2):
                ns = n_g[:, hh:hh + 2, :w]
                zs = z_g[:, hh:hh + 2, :w]
                es = e[:, hh:hh + 2, :w]
                ho = h_next[:, hh:hh + 2, so:]
                if first:
                    nc.vector.tensor_mul(es, zs, ns)
                    nc.vector.tensor_sub(ho, ns, es)
                else:
                    ds = dd[:, hh:hh + 2, :w]
                    nc.vector.tensor_sub(ds, st.h[:, hh:hh + 2, so:], ns)
                    nc.vector.tensor_mul(es, zs, ds)
                    nc.vector.tensor_add(ho, ns, es)
                if not last:
                    h8n = st.h8_next
                    if hh == 0:
                        nc.scalar.activation(h8n[:, hh:hh + 2, nso:],
                                             h_next[:, hh:hh + 2, nso:],
                                             ACT.Copy, scale=1.0)
                    else:
                        nc.vector.tensor_copy(h8n[:, hh:hh + 2, nso:],
                                              h_next[:, hh:hh + 2, nso:])
            st.h = h_next
            st.so = so
            if not last:
                st.h8 = st.h8_next

        def emit_mlp(t, hf, hb):
            hid = gpool.tile([128, 4, S], BF16, tag="e", bufs=2, name="hid")
            for i in range(4):
                ps = xpps.tile([128, 512], F32, tag="xp", name="mps")
                for k in range(8):
                    src = hf if k < 4 else hb
                    nc.tensor.matmul(ps[:], w1[:, k, i * 128:(i + 1) * 128],
                                     src[:, k % 4, :], start=k == 0, stop=k == 7)
                nc.scalar.activation(hid[:, i, :], ps[:], ACT.Relu,
                                     bias=bt[:, 32 + i:33 + i], scale=1.0 / SCALE)
            onats = [opool.tile([128, H], F32, tag="o", name=f"onat{g}")
                     for g in range(4)]
            ob = gpool.tile([128, 4, S], BF16, tag="dd", bufs=2, name="ob")
            for i in range(4):
                ps = xpps.tile([128, 512], F32, tag="xp", name="ops")
                for k in range(4):
                    nc.tensor.matmul(ps[:], w2[:, k, i * 128:(i + 1) * 128],
                                     hid[:, k, :], start=k == 0, stop=k == 3)
                nc.scalar.activation(ob[:, i, :], ps[:], ACT.Identity,
                                     bias=bt[:, 36 + i:37 + i], scale=1.0 / SCALE)
                for g in range(4):
                    tp = ghps.tile([128, 128], BF16, tag="gh", name="tp")
                    nc.tensor.transpose(tp[:], ob[:, i, g * 128:(g + 1) * 128],
                                        ident[:])
                    nc.vector.tensor_copy(onats[g][:, i * 128:(i + 1) * 128],
                                          tp[:])
            for g in range(4):
                r0 = t * S + g * 128
                nc.sync.dma_start(y_d[r0:r0 + 128, :], onats[g][:])


        # All 4 recurrence chains (dir x tile) are independent: interleave
        # them step-by-step, aligned at their final step, so the PE always
        # has the other chains' matmul streams to hide one chain's gate
        # chain (ACT -> DVE -> cast) latency.  z_g/n_g are persistent
        # per-chain tiles (written in phase A, read in phase B) so pool
        # rotation cannot create forward WAR edges across chains; all other
        # gate tiles are consumed within their own phase.
        chains = [(d, t, sched[t][0 if d == 'f' else 1], St())
                  for d in 'fb' for t in range(ntiles)]
        for ci, (d, t, steps, st) in enumerate(chains):
            st.z_g = const.tile([128, 4, S], BF16, name=f"zg{ci}")
            st.n_g = const.tile([128, 4, S], BF16, name=f"ng{ci}")
        # Start-aligned rounds: every chain begins in round 0, so the four
        # narrow first steps stack up to keep the PE fed.  A tile's MLP is
        # emitted as soon as both its chains finish, so tile 0's MLP fills
        # tile 1's late-round gate-chain latency.
        states_f = [c[3] for c in chains[:ntiles]]
        states_b = [c[3] for c in chains[ntiles:]]
        mx = max(len(c[2]) for c in chains)
        mlp_emitted = set()
        for j in range(mx):
            pas = {}
            for ci, (d, t, steps, st) in enumerate(chains):
                if j < len(steps):
                    if j + 1 < len(steps):  # pre-alloc next step's h8 tile
                        st.h8_next = h8pool.tile([128, 4, S], F8,
                                                 tag="h8", name="h8")
                    pas[ci] = phase_a(d, t, steps, j, st)
            for ci, (d, t, steps, st) in enumerate(chains):
                if ci in pas:
                    phase_b(d, t, steps, j, st, pas[ci])
            for t in range(ntiles):
                if t not in mlp_emitted and all(
                        j >= len(sched[t][dd]) - 1 for dd in (0, 1)):
                    emit_mlp(t, states_f[t].h, states_b[t].h)
                    mlp_emitted.add(t)


    nc.compile()
    return nc


def kernel(padded_window, window_len, Wih_f, Whh_f, bih_f, bhh_f,
           Wih_b, Whh_b, bih_b, bhh_b, W1, b1, W2, b2):
    wl = np.asarray(window_len)
    lf = (wl - 1) // 2 + 1
    lb = wl // 2 + 1
    order = np.argsort(wl, kind="stable")

    Bc = B // NCORES
    ntiles = Bc // S
    lf_pc = lf[order].reshape(-1, NCORES)
    lb_pc = lb[order].reshape(-1, NCORES)

    def dir_steps(lens_pc, t):
        seg = lens_pc[t * S:(t + 1) * S]  # [S, NCORES]
        n = int(seg.max())
        steps = []
        for j in range(n):
            need = n - j
            cnt = (seg >= need).sum(axis=0)
            w = int(min(S, max(32, -(-int(cnt.max()) // 32) * 32)))
            masked = bool(cnt.min() < w)
            steps.append((w, masked))
        return tuple(steps)

    sched = tuple((dir_steps(lf_pc, t), dir_steps(lb_pc, t))
                  for t in range(ntiles))

    if sched not in _PROGRAM_CACHE:
        _PROGRAM_CACHE[sched] = _build_program(sched)
    nc = _PROGRAM_CACHE[sched]

    f32, bf16, f8 = np.float32, ml_dtypes.bfloat16, ml_dtypes.float8_e4m3
    sc = np.float32(SCALE)

    def q8(a):
        return np.asarray(a, f32).astype(f8)

    wnf = np.ascontiguousarray(sc * Wih_f[2 * H:].T).astype(bf16)
    wnb = np.ascontiguousarray(sc * Wih_b[2 * H:].T).astype(bf16)
    wrz8f = q8(sc * np.ascontiguousarray(Wih_f[:2 * H].T))
    wrz8b = q8(sc * np.ascontiguousarray(Wih_b[:2 * H].T))
    wh8f = q8(sc * np.ascontiguousarray(Whh_f.T))
    wh8b = q8(sc * np.ascontiguousarray(Whh_b.T))
    w1 = (sc * np.ascontiguousarray(W1.T)).astype(bf16)
    w2 = (sc * np.ascontiguousarray(W2.T)).astype(bf16)

    def chunks(v):  # [512] -> [4, 128]
        return np.asarray(v, f32).reshape(4, 128)

    bias = np.concatenate([
        chunks((bih_f + bhh_f)[:H]), chunks((bih_f + bhh_f)[H:2 * H]),
        chunks(sc * bhh_f[2 * H:]), chunks(bih_f[2 * H:]),
        chunks((bih_b + bhh_b)[:H]), chunks((bih_b + bhh_b)[H:2 * H]),
        chunks(sc * bhh_b[2 * H:]), chunks(bih_b[2 * H:]),
        chunks(b1), chunks(b2),
    ], 0)  # [40, 128]

    pw = np.asarray(padded_window, f32)
    in_maps = []
    p8 = np.arange(8)
    for c in range(NCORES):
        idx = order[c::NCORES]
        xT = np.ascontiguousarray(pw[idx].transpose(1, 2, 0))  # [15, 512, Bc]
        mzf = (p8[:, None] < (8 - lf[idx])[None, :]).astype(bf16)
        mzb = (p8[:, None] < (8 - lb[idx])[None, :]).astype(bf16)
        in_maps.append({
            "xb": xT.astype(bf16), "x8": xT.astype(f8),
            "wnf": wnf, "wnb": wnb, "wrz8f": wrz8f, "wrz8b": wrz8b,
            "wh8f": wh8f, "wh8b": wh8b, "w1": w1, "w2": w2,
            "bias": bias, "maskzf": mzf, "maskzb": mzb,
        })

    trace = bool(os.environ.get("GRU_TRACE"))
    kw = {}
    if os.environ.get("GRU_TMPDIR"):
        kw["tmpdir"] = os.environ["GRU_TMPDIR"]
    res = run_bass_kernel_spmd(nc, in_maps, core_ids=list(range(NCORES)),
                               trace=trace, **kw)
    global LAST_RESULT
    LAST_RESULT = res
    out = np.empty((B, H), f32)
    for c in range(NCORES):
        out[order[c::NCORES]] = res.results[c]["y"]
    return out


# revision 37
# speedup vs baseline: 1.0029x; 1.0029x over previous
"""BiGRU encoder kernel for 8 Trainium2 NeuronCores.

Strategy (v2 — mixed precision bf16/fp8):
  - Masked GRUs over fixed position ranges (see v1): forward runs positions
    ascending into the center, backward descending; a sample of length l only
    starts updating z-steps from its start; pre-start samples are forced to
    z==1 (h'==h==0 exactly) by adding +SCALE*BIG to the z-gate preactivation.
  - Sort samples by window_len, deal round-robin to 8 cores; per core two
    batch tiles of 512 sorted samples; per GRU step only the suffix of
    samples that needs the step is processed (width gran 32).
  - Precision: z,n input projections in bf16 (accuracy-critical), r input
    projection and ALL hidden projections in fp8e4m3 with DoubleRow matmuls
    (2 k-chunks per instruction) — GRU recurrence is contractive, so fp8
    noise on the hidden path washes out (measured 1.0e-2 vs 2e-2 budget).
    All weights pre-scaled x16 host-side (fp8 subnormal floor), undone by
    the activation `scale=1/16`.
  - z-mask applied as a rank-1 matmul (stationary [1,128]=640, moving 0/1
    row) accumulated into the z PSUM group — no DVE mask add.
  - n-gate: tt=(ghn+16*bhh_n)*r on DVE (bf16 out), then an identity matmul
    accumulates tt into the xn PSUM group — no separate ss add.
  - h carried in bf16; cast to fp8 (scalar engine) for the next hidden
    matmul.  h-update chain (dd,e,ho) runs merged over 2-kchunk halves in
    bf16 (DVE 2x mode), halves staggered so the next step's first DoubleRow
    pair can start after half 0.
  - The two batch tiles are interleaved step-by-step, and each step is
    emitted in two phases (A: DMAs+projection matmuls+r/z ACTs+tt, B:
    id-matmul+n ACT+h update+cast) so the PE always has the other tile's
    matmul stream to chew while one tile's gate chain drains.
"""

import os
from contextlib import ExitStack

import numpy as np
import ml_dtypes

import concourse.bacc as bacc
import concourse.tile as tile
from concourse import mybir
from concourse.bass_utils import run_bass_kernel_spmd
from concourse.masks import make_identity

NCORES = 8
B, T, D, H = 8192, 15, 512, 512
G = 3 * H
BIG = 40.0
SCALE = 16.0
S = 512  # samples per batch tile
F32 = mybir.dt.float32
BF16 = mybir.dt.bfloat16
F8 = mybir.dt.float8e4
DR_MIN = int(os.environ.get("GRU_DRMIN", "320"))  # min width for DoubleRow

ACT = mybir.ActivationFunctionType
ALU = mybir.AluOpType
DRM = mybir.MatmulPerfMode.DoubleRow

_PROGRAM_CACHE = {}
LAST_RESULT = None


def _build_program(sched):
    """sched: per tile, (f_steps, b_steps); each step = (width, masked)."""
    ntiles = len(sched)
    Bc = S * ntiles
    nc = bacc.Bacc("TRN2", target_bir_lowering=False, debug=False,
                   num_devices=NCORES)

    xb_d = nc.dram_tensor("xb", [T, D, Bc], BF16, kind="ExternalInput")
    x8_d = nc.dram_tensor("x8", [T, D, Bc], F8, kind="ExternalInput")
    wzn_d = {'f': nc.dram_tensor("wnf", [D, H], BF16, kind="ExternalInput"),
             'b': nc.dram_tensor("wnb", [D, H], BF16, kind="ExternalInput")}
    wr8_d = {'f': nc.dram_tensor("wrz8f", [D, 2 * H], F8, kind="ExternalInput"),
             'b': nc.dram_tensor("wrz8b", [D, 2 * H], F8, kind="ExternalInput")}
    wh8_d = {'f': nc.dram_tensor("wh8f", [H, G], F8, kind="ExternalInput"),
             'b': nc.dram_tensor("wh8b", [H, G], F8, kind="ExternalInput")}
    w1_d = nc.dram_tensor("w1", [2 * H, H], BF16, kind="ExternalInput")
    w2_d = nc.dram_tensor("w2", [H, H], BF16, kind="ExternalInput")
    bias_d = nc.dram_tensor("bias", [40, 128], F32, kind="ExternalInput")
    m_d = {'f': nc.dram_tensor("maskzf", [8, Bc], BF16, kind="ExternalInput"),
           'b': nc.dram_tensor("maskzb", [8, Bc], BF16, kind="ExternalInput")}
    y_d = nc.dram_tensor("y", [Bc, H], F32, kind="ExternalOutput")

    with tile.TileContext(nc) as tc, ExitStack() as ctx:
        const = ctx.enter_context(tc.tile_pool(name="const", bufs=1))
        xpool = ctx.enter_context(tc.tile_pool(name="x", bufs=4))
        hpool = ctx.enter_context(tc.tile_pool(name="h", bufs=6))
        h8pool = ctx.enter_context(tc.tile_pool(name="h8", bufs=4))
        gpool = ctx.enter_context(tc.tile_pool(name="g", bufs=2))
        opool = ctx.enter_context(tc.tile_pool(name="o", bufs=4))
        rzps = ctx.enter_context(tc.tile_pool(name="rz", bufs=4, space="PSUM"))
        xpps = ctx.enter_context(tc.tile_pool(name="xp", bufs=2, space="PSUM"))
        ghps = ctx.enter_context(tc.tile_pool(name="gh", bufs=2, space="PSUM"))

        def load_w(dram, kchunks, cols, dt, name, eng):
            t_ = const.tile([128, kchunks, cols], dt, name=name)
            src = dram.rearrange("(c k) g -> k c g", k=128)
            for c in range(kchunks):
                eng.dma_start(t_[:, c, :], src[:, c, :])
            return t_

        # per-kchunk DMAs interleaved across both directions and two queues
        # so every chain's first-needed weight chunks land within ~3us
        wzn, wr8, wh8 = {}, {}, {}
        for d in 'fb':
            wzn[d] = const.tile([128, 4, H], BF16, name=f"wn{d}")
            wr8[d] = const.tile([128, 4, 2 * H], F8, name=f"wrz8{d}")
            wh8[d] = const.tile([128, 4, G], F8, name=f"wh8{d}")
        for c in range(4):
            for d, eng in (('f', nc.scalar), ('b', nc.scalar)):
                for t_, dram in ((wzn[d], wzn_d[d]), (wr8[d], wr8_d[d]),
                                 (wh8[d], wh8_d[d])):
                    src_ = dram.rearrange("(c k) g -> k c g", k=128)
                    eng.dma_start(t_[:, c, :], src_[:, c, :])
        w1 = load_w(w1_d, 8, H, BF16, "w1", nc.gpsimd)
        w2 = load_w(w2_d, 4, H, BF16, "w2", nc.gpsimd)
        bt = const.tile([128, 40], F32)
        nc.gpsimd.dma_start(bt[:], bias_d.rearrange("n p -> p n"))
        ident = const.tile([128, 128], BF16)
        make_identity(nc, ident[:])

        class St:  # per (dir, tile) recurrence state
            h = None
            h8 = None
            so = None

        def phase_a(d, t, steps, j, st):
            """DMAs, projection matmuls, mask, r/z ACTs, tt."""
            w, masked = steps[j]
            nsteps = len(steps)
            first = j == 0
            so = S - w
            s0 = t * S
            a0 = s0 + so
            bb = 0 if d == 'f' else 16
            pos = (8 - nsteps + j) if d == 'f' else (6 + nsteps - j)
            use_dr = w >= DR_MIN

            mt = None
            if masked:
                mt = gpool.tile([128, S], BF16, tag="mt", bufs=2, name="mt")
                nc.gpsimd.dma_start(
                    mt[:, :w],
                    m_d[d][8 - nsteps + j, a0:s0 + S].partition_broadcast(128))
            xt = xpool.tile([128, 4, S], BF16, tag="x", name="xt")
            nc.sync.dma_start(
                xt[:, :, :w],
                xb_d[pos].rearrange("(c k) s -> k c s", k=128)[:, :, a0:s0 + S])
            x8 = xpool.tile([128, 4, S], F8, tag="x8", bufs=4, name="x8t")
            nc.sync.dma_start(
                x8[:, :, :w],
                x8_d[pos].rearrange("(c k) s -> k c s", k=128)[:, :, a0:s0 + S])

            rps, zps, xpn, ghn = [], [], [], []
            for i in range(4):
                rps.append(rzps.tile([128, 512], F32, tag="rz", name=f"rps{i}"))
                zps.append(rzps.tile([128, 512], F32, tag="rz", name=f"zps{i}"))
                xpn.append(xpps.tile([128, 512], F32, tag="xp", name=f"xpn{i}"))
                if not first:
                    ghn.append(ghps.tile([128, 512], F32, tag="gh",
                                         name=f"ghn{i}"))
            wn, wh, wrz = wzn[d], wh8[d], wr8[d]
            for i in range(4):
                c0, c1 = i * 128, (i + 1) * 128
                # n input projection (bf16); group closes here, ss is on DVE
                for k in range(4):
                    nc.tensor.matmul(xpn[i][:, :w], wn[:, k, c0:c1],
                                     xt[:, k, :w], start=k == 0, stop=k == 3)
                # r and z input projections (fp8)
                if use_dr:
                    for kp in (0, 2):
                        nc.tensor.matmul(rps[i][:, :w], wrz[:, kp:kp + 2, c0:c1],
                                         x8[:, kp:kp + 2, :w], start=kp == 0,
                                         stop=first and kp == 2, perf_mode=DRM)
                        nc.tensor.matmul(zps[i][:, :w],
                                         wrz[:, kp:kp + 2, H + c0:H + c1],
                                         x8[:, kp:kp + 2, :w], start=kp == 0,
                                         stop=first and kp == 2,
                                         perf_mode=DRM)
                else:
                    for k in range(4):
                        nc.tensor.matmul(rps[i][:, :w], wrz[:, k, c0:c1],
                                         x8[:, k, :w], start=k == 0,
                                         stop=first and k == 3)
                        nc.tensor.matmul(zps[i][:, :w], wrz[:, k, H + c0:H + c1],
                                         x8[:, k, :w], start=k == 0,
                                         stop=first and k == 3)
                # hidden projections (fp8), accumulate into same groups
                if not first:
                    h8 = st.h8
                    if use_dr:
                        for kp in (0, 2):
                            hk = h8[:, kp:kp + 2, so:]
                            last = kp == 2
                            nc.tensor.matmul(rps[i][:, :w], wh[:, kp:kp + 2, c0:c1],
                                             hk, start=False, stop=last,
                                             perf_mode=DRM)
                            nc.tensor.matmul(zps[i][:, :w],
                                             wh[:, kp:kp + 2, H + c0:H + c1],
                                             hk, start=False, stop=last,
                                             perf_mode=DRM)
                            nc.tensor.matmul(ghn[i][:, :w],
                                             wh[:, kp:kp + 2, 2 * H + c0:2 * H + c1],
                                             hk, start=kp == 0, stop=last,
                                             perf_mode=DRM)
                    else:
                        for k in range(4):
                            hk = st.h8[:, k, so:]
                            last = k == 3
                            nc.tensor.matmul(rps[i][:, :w], wh[:, k, c0:c1],
                                             hk, start=False, stop=last)
                            nc.tensor.matmul(zps[i][:, :w], wh[:, k, H + c0:H + c1],
                                             hk, start=False, stop=last)
                            nc.tensor.matmul(ghn[i][:, :w],
                                             wh[:, k, 2 * H + c0:2 * H + c1],
                                             hk, start=k == 0, stop=last)
            r_g = gpool.tile([128, 4, S], BF16, tag="r", bufs=2, name="r_g")
            tt = gpool.tile([128, 4, S], BF16, tag="tt", bufs=2, name="tt")
            z_g, n_g = st.z_g, st.n_g
            for i in range(4):
                nc.scalar.activation(r_g[:, i, :w], rps[i][:, :w], ACT.Sigmoid,
                                     bias=bt[:, bb + i:bb + i + 1],
                                     scale=1.0 / SCALE)
                if masked:
                    zr = gpool.tile([128, 512], BF16, tag="zr", bufs=2,
                                    name="zr")
                    nc.scalar.activation(zr[:, :w], zps[i][:, :w], ACT.Sigmoid,
                                         bias=bt[:, bb + 4 + i:bb + 5 + i],
                                         scale=1.0 / SCALE)
                    nc.vector.tensor_max(z_g[:, i, :w], zr[:, :w], mt[:, :w])
                else:
                    nc.scalar.activation(z_g[:, i, :w], zps[i][:, :w],
                                         ACT.Sigmoid,
                                         bias=bt[:, bb + 4 + i:bb + 5 + i],
                                         scale=1.0 / SCALE)
                if first:
                    nc.vector.tensor_scalar_mul(tt[:, i, :w], r_g[:, i, :w],
                                                bt[:, bb + 8 + i:bb + 9 + i])
                else:
                    nc.vector.scalar_tensor_tensor(
                        tt[:, i, :w], ghn[i][:, :w],
                        bt[:, bb + 8 + i:bb + 9 + i], r_g[:, i, :w],
                        op0=ALU.add, op1=ALU.mult)
                # ss and the n ACT stay in phase A so the xpn bank and the
                # ss buffer are both freed within the phase (no cross-phase
                # PSUM/buffer WAR edges -> no scheduling deadlock)
                ss = gpool.tile([128, 512], F32, tag="ss", bufs=4, name="ss")
                nc.vector.tensor_add(ss[:, :w], tt[:, i, :w], xpn[i][:, :w])
                nc.scalar.activation(n_g[:, i, :w], ss[:, :w], ACT.Tanh,
                                     bias=bt[:, bb + 12 + i:bb + 13 + i],
                                     scale=1.0 / SCALE)
            return w, so

        def phase_b(d, t, steps, j, st, pa):
            """h update, fp8 cast."""
            w, so = pa
            z_g, n_g = st.z_g, st.n_g
            nsteps = len(steps)
            first = j == 0
            last = j == nsteps - 1
            h_next = hpool.tile([128, 4, S], BF16, tag="hf" if last else "h",
                                bufs=4 if last else 6, name="h")
            if not last:
                nw = steps[j + 1][0]
                nso = S - nw
                if nso < so:
                    nc.gpsimd.memset(h_next[:, :, nso:so], 0.0)
            e = gpool.tile([128, 4, S], BF16, tag="e", bufs=2, name="e")
            dd = None
            if not first:
                dd = gpool.tile([128, 4, S], BF16, tag="dd", bufs=2, name="dd")
            for hh in (0, 2):
                ns = n_g[:, hh:hh + 2, :w]
                zs = z_g[:, hh:hh + 2, :w]
                es = e[:, hh:hh + 2, :w]
                ho = h_next[:, hh:hh + 2, so:]
                if first:
                    nc.vector.tensor_mul(es, zs, ns)
                    nc.vector.tensor_sub(ho, ns, es)
                else:
                    ds = dd[:, hh:hh + 2, :w]
                    nc.vector.tensor_sub(ds, st.h[:, hh:hh + 2, so:], ns)
                    nc.vector.tensor_mul(es, zs, ds)
                    nc.vector.tensor_add(ho, ns, es)
                if not last:
                    h8n = st.h8_next
                    if hh == 0:
                        nc.scalar.activation(h8n[:, hh:hh + 2, nso:],
                                             h_next[:, hh:hh + 2, nso:],
                                             ACT.Copy, scale=1.0)
                    else:
                        nc.vector.tensor_copy(h8n[:, hh:hh + 2, nso:],
                                              h_next[:, hh:hh + 2, nso:])
            st.h = h_next
            st.so = so
            if not last:
                st.h8 = st.h8_next

        # All 4 recurrence chains (dir x tile) are independent: interleave
        # them step-by-step, aligned at their final step, so the PE always
        # has the other chains' matmul streams to hide one chain's gate
        # chain (ACT -> DVE -> cast) latency.  z_g/n_g are persistent
        # per-chain tiles (written in phase A, read in phase B) so pool
        # rotation cannot create forward WAR edges across chains; all other
        # gate tiles are consumed within their own phase.
        # tile-major chain order: both of tile 0's chains finish their
        # final-round B phases while tile 1's A matmuls still stream, so
        # the tile-0 MLP starts without waiting on tile 1's gate chains
        chains = [(d, t, sched[t][0 if d == 'f' else 1], St())
                  for t in range(ntiles) for d in 'fb']
        for ci, (d, t, steps, st) in enumerate(chains):
            st.z_g = const.tile([128, 4, S], BF16, name=f"zg{ci}")
            st.n_g = const.tile([128, 4, S], BF16, name=f"ng{ci}")
        mx = max(len(c[2]) for c in chains)
        for k in range(mx, 0, -1):
            pas = {}
            for ci, (d, t, steps, st) in enumerate(chains):
                if len(steps) >= k:
                    if k > 1:  # next step exists: pre-alloc its h8 tile
                        st.h8_next = h8pool.tile([128, 4, S], F8,
                                                 tag="h8", name="h8")
                    pas[ci] = phase_a(d, t, steps, len(steps) - k, st)
            for ci, (d, t, steps, st) in enumerate(chains):
                if ci in pas:
                    phase_b(d, t, steps, len(steps) - k, st, pas[ci])

        states_f = [c[3] for c in chains if c[0] == 'f']
        states_b = [c[3] for c in chains if c[0] == 'b']

        def emit_mlp(t, hf, hb):
            hid = gpool.tile([128, 4, S], BF16, tag="e", bufs=2, name="hid")
            for i in range(4):
                ps = xpps.tile([128, 512], F32, tag="xp", name="mps")
                for k in range(8):
                    src = hf if k < 4 else hb
                    nc.tensor.matmul(ps[:], w1[:, k, i * 128:(i + 1) * 128],
                                     src[:, k % 4, :], start=k == 0, stop=k == 7)
                nc.scalar.activation(hid[:, i, :], ps[:], ACT.Relu,
                                     bias=bt[:, 32 + i:33 + i], scale=1.0 / SCALE)
            onats = [opool.tile([128, H], F32, tag="o", name=f"onat{g}")
                     for g in range(4)]
            ob = gpool.tile([128, 4, S], BF16, tag="dd", bufs=2, name="ob")
            for i in range(4):
                ps = xpps.tile([128, 512], F32, tag="xp", name="ops")
                for k in range(4):
                    nc.tensor.matmul(ps[:], w2[:, k, i * 128:(i + 1) * 128],
                                     hid[:, k, :], start=k == 0, stop=k == 3)
                nc.scalar.activation(ob[:, i, :], ps[:], ACT.Identity,
                                     bias=bt[:, 36 + i:37 + i], scale=1.0 / SCALE)
                for g in range(4):
                    tp = ghps.tile([128, 128], BF16, tag="gh", name="tp")
                    nc.tensor.transpose(tp[:], ob[:, i, g * 128:(g + 1) * 128],
                                        ident[:])
                    nc.vector.tensor_copy(onats[g][:, i * 128:(i + 1) * 128],
                                          tp[:])
            for g in range(4):
                r0 = t * S + g * 128
                nc.sync.dma_start(y_d[r0:r0 + 128, :], onats[g][:])

        for t in range(ntiles):
            emit_mlp(t, states_f[t].h, states_b[t].h)

    nc.compile()
    return nc


def kernel(padded_window, window_len, Wih_f, Whh_f, bih_f, bhh_f,
           Wih_b, Whh_b, bih_b, bhh_b, W1, b1, W2, b2):
    wl = np.asarray(window_len)
    lf = (wl - 1) // 2 + 1
    lb = wl // 2 + 1
    order = np.argsort(wl, kind="stable")

    Bc = B // NCORES
    ntiles = Bc // S
    lf_pc = lf[order].reshape(-1, NCORES)
    lb_pc = lb[order].reshape(-1, NCORES)

    def dir_steps(lens_pc, t):
        seg = lens_pc[t * S:(t + 1) * S]  # [S, NCORES]
        n = int(seg.max())
        steps = []
        for j in range(n):
            need = n - j
            cnt = (seg >= need).sum(axis=0)
            w = int(min(S, max(32, -(-int(cnt.max()) // 32) * 32)))
            masked = bool(cnt.min() < w)
            steps.append((w, masked))
        return tuple(steps)

    sched = tuple((dir_steps(lf_pc, t), dir_steps(lb_pc, t))
                  for t in range(ntiles))

    if sched not in _PROGRAM_CACHE:
        _PROGRAM_CACHE[sched] = _build_program(sched)
    nc = _PROGRAM_CACHE[sched]

    f32, bf16, f8 = np.float32, ml_dtypes.bfloat16, ml_dtypes.float8_e4m3
    sc = np.float32(SCALE)

    def q8(a):
        return np.asarray(a, f32).astype(f8)

    wnf = np.ascontiguousarray(sc * Wih_f[2 * H:].T).astype(bf16)
    wnb = np.ascontiguousarray(sc * Wih_b[2 * H:].T).astype(bf16)
    wrz8f = q8(sc * np.ascontiguousarray(Wih_f[:2 * H].T))
    wrz8b = q8(sc * np.ascontiguousarray(Wih_b[:2 * H].T))
    wh8f = q8(sc * np.ascontiguousarray(Whh_f.T))
    wh8b = q8(sc * np.ascontiguousarray(Whh_b.T))
    w1 = (sc * np.ascontiguousarray(W1.T)).astype(bf16)
    w2 = (sc * np.ascontiguousarray(W2.T)).astype(bf16)

    def chunks(v):  # [512] -> [4, 128]
        return np.asarray(v, f32).reshape(4, 128)

    bias = np.concatenate([
        chunks((bih_f + bhh_f)[:H]), chunks((bih_f + bhh_f)[H:2 * H]),
        chunks(sc * bhh_f[2 * H:]), chunks(bih_f[2 * H:]),
        chunks((bih_b + bhh_b)[:H]), chunks((bih_b + bhh_b)[H:2 * H]),
        chunks(sc * bhh_b[2 * H:]), chunks(bih_b[2 * H:]),
        chunks(b1), chunks(b2),
    ], 0)  # [40, 128]

    pw = np.asarray(padded_window, f32)
    in_maps = []
    p8 = np.arange(8)
    for c in range(NCORES):
        idx = order[c::NCORES]
        xT = np.ascontiguousarray(pw[idx].transpose(1, 2, 0))  # [15, 512, Bc]
        mzf = (p8[:, None] < (8 - lf[idx])[None, :]).astype(bf16)
        mzb = (p8[:, None] < (8 - lb[idx])[None, :]).astype(bf16)
        in_maps.append({
            "xb": xT.astype(bf16), "x8": xT.astype(f8),
            "wnf": wnf, "wnb": wnb, "wrz8f": wrz8f, "wrz8b": wrz8b,
            "wh8f": wh8f, "wh8b": wh8b, "w1": w1, "w2": w2,
            "bias": bias, "maskzf": mzf, "maskzb": mzb,
        })

    trace = bool(os.environ.get("GRU_TRACE"))
    kw = {}
    if os.environ.get("GRU_TMPDIR"):
        kw["tmpdir"] = os.environ["GRU_TMPDIR"]
    res = run_bass_kernel_spmd(nc, in_maps, core_ids=list(range(NCORES)),
                               trace=trace, **kw)
    global LAST_RESULT
    LAST_RESULT = res
    out = np.empty((B, H), f32)
    for c in range(NCORES):
        out[order[c::NCORES]] = res.results[c]["y"]
    return out


# revision 38
# speedup vs baseline: 1.0107x; 1.0078x over previous
"""BiGRU encoder kernel for 8 Trainium2 NeuronCores.

Strategy (v2 — mixed precision bf16/fp8):
  - Masked GRUs over fixed position ranges (see v1): forward runs positions
    ascending into the center, backward descending; a sample of length l only
    starts updating z-steps from its start; pre-start samples are forced to
    z==1 (h'==h==0 exactly) by adding +SCALE*BIG to the z-gate preactivation.
  - Sort samples by window_len, deal round-robin to 8 cores; per core two
    batch tiles of 512 sorted samples; per GRU step only the suffix of
    samples that needs the step is processed (width gran 32).
  - Precision: z,n input projections in bf16 (accuracy-critical), r input
    projection and ALL hidden projections in fp8e4m3 with DoubleRow matmuls
    (2 k-chunks per instruction) — GRU recurrence is contractive, so fp8
    noise on the hidden path washes out (measured 1.0e-2 vs 2e-2 budget).
    All weights pre-scaled x16 host-side (fp8 subnormal floor), undone by
    the activation `scale=1/16`.
  - z-mask applied as a rank-1 matmul (stationary [1,128]=640, moving 0/1
    row) accumulated into the z PSUM group — no DVE mask add.
  - n-gate: tt=(ghn+16*bhh_n)*r on DVE (bf16 out), then an identity matmul
    accumulates tt into the xn PSUM group — no separate ss add.
  - h carried in bf16; cast to fp8 (scalar engine) for the next hidden
    matmul.  h-update chain (dd,e,ho) runs merged over 2-kchunk halves in
    bf16 (DVE 2x mode), halves staggered so the next step's first DoubleRow
    pair can start after half 0.
  - The two batch tiles are interleaved step-by-step, and each step is
    emitted in two phases (A: DMAs+projection matmuls+r/z ACTs+tt, B:
    id-matmul+n ACT+h update+cast) so the PE always has the other tile's
    matmul stream to chew while one tile's gate chain drains.
"""

import os
from contextlib import ExitStack

import numpy as np
import ml_dtypes

import concourse.bacc as bacc
import concourse.tile as tile
from concourse import mybir
from concourse.bass_utils import run_bass_kernel_spmd
from concourse.masks import make_identity

NCORES = 8
B, T, D, H = 8192, 15, 512, 512
G = 3 * H
BIG = 40.0
SCALE = 16.0
S = 512  # samples per batch tile
F32 = mybir.dt.float32
BF16 = mybir.dt.bfloat16
F8 = mybir.dt.float8e4
DR_MIN = int(os.environ.get("GRU_DRMIN", "320"))  # min width for DoubleRow

ACT = mybir.ActivationFunctionType
ALU = mybir.AluOpType
DRM = mybir.MatmulPerfMode.DoubleRow

_PROGRAM_CACHE = {}
LAST_RESULT = None


def _build_program(sched):
    """sched: per tile, (f_steps, b_steps); each step = (width, masked)."""
    ntiles = len(sched)
    Bc = S * ntiles
    nc = bacc.Bacc("TRN2", target_bir_lowering=False, debug=False,
                   num_devices=NCORES)

    xb_d = nc.dram_tensor("xb", [T, D, Bc], BF16, kind="ExternalInput")
    x8_d = nc.dram_tensor("x8", [T, D, Bc], F8, kind="ExternalInput")
    wzn_d = {'f': nc.dram_tensor("wnf", [D, H], BF16, kind="ExternalInput"),
             'b': nc.dram_tensor("wnb", [D, H], BF16, kind="ExternalInput")}
    wr8_d = {'f': nc.dram_tensor("wrz8f", [D, 2 * H], F8, kind="ExternalInput"),
             'b': nc.dram_tensor("wrz8b", [D, 2 * H], F8, kind="ExternalInput")}
    wh8_d = {'f': nc.dram_tensor("wh8f", [H, G], F8, kind="ExternalInput"),
             'b': nc.dram_tensor("wh8b", [H, G], F8, kind="ExternalInput")}
    w1_d = nc.dram_tensor("w1", [2 * H, H], BF16, kind="ExternalInput")
    w2_d = nc.dram_tensor("w2", [H, H], BF16, kind="ExternalInput")
    bias_d = nc.dram_tensor("bias", [40, 128], F32, kind="ExternalInput")
    m_d = {'f': nc.dram_tensor("maskzf", [8, Bc], BF16, kind="ExternalInput"),
           'b': nc.dram_tensor("maskzb", [8, Bc], BF16, kind="ExternalInput")}
    y_d = nc.dram_tensor("y", [Bc, H], F32, kind="ExternalOutput")

    with tile.TileContext(nc) as tc, ExitStack() as ctx:
        const = ctx.enter_context(tc.tile_pool(name="const", bufs=1))
        xpool = ctx.enter_context(tc.tile_pool(name="x", bufs=4))
        hpool = ctx.enter_context(tc.tile_pool(name="h", bufs=6))
        h8pool = ctx.enter_context(tc.tile_pool(name="h8", bufs=4))
        gpool = ctx.enter_context(tc.tile_pool(name="g", bufs=2))
        opool = ctx.enter_context(tc.tile_pool(name="o", bufs=4))
        rzps = ctx.enter_context(tc.tile_pool(name="rz", bufs=4, space="PSUM"))
        xpps = ctx.enter_context(tc.tile_pool(name="xp", bufs=2, space="PSUM"))
        ghps = ctx.enter_context(tc.tile_pool(name="gh", bufs=2, space="PSUM"))

        def load_w(dram, kchunks, cols, dt, name, eng):
            t_ = const.tile([128, kchunks, cols], dt, name=name)
            src = dram.rearrange("(c k) g -> k c g", k=128)
            for c in range(kchunks):
                eng.dma_start(t_[:, c, :], src[:, c, :])
            return t_

        # per-kchunk DMAs interleaved across both directions and two queues
        # so every chain's first-needed weight chunks land within ~3us
        wzn, wr8, wh8 = {}, {}, {}
        for d in 'fb':
            wzn[d] = const.tile([128, 4, H], BF16, name=f"wn{d}")
            wr8[d] = const.tile([128, 4, 2 * H], F8, name=f"wrz8{d}")
            wh8[d] = const.tile([128, 4, G], F8, name=f"wh8{d}")
        for c in range(4):
            for d, eng in (('f', nc.scalar), ('b', nc.scalar)):
                for t_, dram in ((wzn[d], wzn_d[d]), (wr8[d], wr8_d[d]),
                                 (wh8[d], wh8_d[d])):
                    src_ = dram.rearrange("(c k) g -> k c g", k=128)
                    eng.dma_start(t_[:, c, :], src_[:, c, :])
        w1 = load_w(w1_d, 8, H, BF16, "w1", nc.gpsimd)
        w2 = load_w(w2_d, 4, H, BF16, "w2", nc.gpsimd)
        bt = const.tile([128, 40], F32)
        nc.gpsimd.dma_start(bt[:], bias_d.rearrange("n p -> p n"))
        ident = const.tile([128, 128], BF16)
        make_identity(nc, ident[:])

        class St:  # per (dir, tile) recurrence state
            h = None
            h8 = None
            so = None

        def phase_a(d, t, steps, j, st):
            """DMAs, projection matmuls, mask, r/z ACTs, tt."""
            w, masked = steps[j]
            nsteps = len(steps)
            first = j == 0
            so = S - w
            s0 = t * S
            a0 = s0 + so
            bb = 0 if d == 'f' else 16
            pos = (8 - nsteps + j) if d == 'f' else (6 + nsteps - j)
            use_dr = w >= DR_MIN

            mt = None
            if masked:
                mt = gpool.tile([128, S], BF16, tag="mt", bufs=2, name="mt")
                nc.gpsimd.dma_start(
                    mt[:, :w],
                    m_d[d][8 - nsteps + j, a0:s0 + S].partition_broadcast(128))
            xt = xpool.tile([128, 4, S], BF16, tag="x", name="xt")
            nc.sync.dma_start(
                xt[:, :, :w],
                xb_d[pos].rearrange("(c k) s -> k c s", k=128)[:, :, a0:s0 + S])
            x8 = xpool.tile([128, 4, S], F8, tag="x8", bufs=4, name="x8t")
            nc.sync.dma_start(
                x8[:, :, :w],
                x8_d[pos].rearrange("(c k) s -> k c s", k=128)[:, :, a0:s0 + S])

            rps, zps, xpn, ghn = [], [], [], []
            for i in range(4):
                rps.append(rzps.tile([128, 512], F32, tag="rz", name=f"rps{i}"))
                zps.append(rzps.tile([128, 512], F32, tag="rz", name=f"zps{i}"))
                xpn.append(xpps.tile([128, 512], F32, tag="xp", name=f"xpn{i}"))
                if not first:
                    ghn.append(ghps.tile([128, 512], F32, tag="gh",
                                         name=f"ghn{i}"))
            wn, wh, wrz = wzn[d], wh8[d], wr8[d]
            for i in range(4):
                c0, c1 = i * 128, (i + 1) * 128
                # n input projection (bf16); group closes here, ss is on DVE
                for k in range(4):
                    nc.tensor.matmul(xpn[i][:, :w], wn[:, k, c0:c1],
                                     xt[:, k, :w], start=k == 0, stop=k == 3)
                # r and z input projections (fp8)
                if use_dr:
                    for kp in (0, 2):
                        nc.tensor.matmul(rps[i][:, :w], wrz[:, kp:kp + 2, c0:c1],
                                         x8[:, kp:kp + 2, :w], start=kp == 0,
                                         stop=first and kp == 2, perf_mode=DRM)
                        nc.tensor.matmul(zps[i][:, :w],
                                         wrz[:, kp:kp + 2, H + c0:H + c1],
                                         x8[:, kp:kp + 2, :w], start=kp == 0,
                                         stop=first and kp == 2,
                                         perf_mode=DRM)
                else:
                    for k in range(4):
                        nc.tensor.matmul(rps[i][:, :w], wrz[:, k, c0:c1],
                                         x8[:, k, :w], start=k == 0,
                                         stop=first and k == 3)
                        nc.tensor.matmul(zps[i][:, :w], wrz[:, k, H + c0:H + c1],
                                         x8[:, k, :w], start=k == 0,
                                         stop=first and k == 3)
                # hidden projections (fp8), accumulate into same groups
                if not first:
                    h8 = st.h8
                    if use_dr:
                        for kp in (0, 2):
                            hk = h8[:, kp:kp + 2, so:]
                            last = kp == 2
                            nc.tensor.matmul(rps[i][:, :w], wh[:, kp:kp + 2, c0:c1],
                                             hk, start=False, stop=last,
                                             perf_mode=DRM)
                            nc.tensor.matmul(zps[i][:, :w],
                                             wh[:, kp:kp + 2, H + c0:H + c1],
                                             hk, start=False, stop=last,
                                             perf_mode=DRM)
                            nc.tensor.matmul(ghn[i][:, :w],
                                             wh[:, kp:kp + 2, 2 * H + c0:2 * H + c1],
                                             hk, start=kp == 0, stop=last,
                                             perf_mode=DRM)
                    else:
                        for k in range(4):
                            hk = st.h8[:, k, so:]
                            last = k == 3
                            nc.tensor.matmul(rps[i][:, :w], wh[:, k, c0:c1],
                                             hk, start=False, stop=last)
                            nc.tensor.matmul(zps[i][:, :w], wh[:, k, H + c0:H + c1],
                                             hk, start=False, stop=last)
                            nc.tensor.matmul(ghn[i][:, :w],
                                             wh[:, k, 2 * H + c0:2 * H + c1],
                                             hk, start=k == 0, stop=last)
            r_g = gpool.tile([128, 4, S], BF16, tag="r", bufs=2, name="r_g")
            tt = gpool.tile([128, 4, S], BF16, tag="tt", bufs=2, name="tt")
            z_g, n_g = st.z_g, st.n_g
            for i in range(4):
                nc.scalar.activation(r_g[:, i, :w], rps[i][:, :w], ACT.Sigmoid,
                                     bias=bt[:, bb + i:bb + i + 1],
                                     scale=1.0 / SCALE)
                if masked:
                    zr = gpool.tile([128, 512], BF16, tag="zr", bufs=2,
                                    name="zr")
                    nc.scalar.activation(zr[:, :w], zps[i][:, :w], ACT.Sigmoid,
                                         bias=bt[:, bb + 4 + i:bb + 5 + i],
                                         scale=1.0 / SCALE)
                    nc.vector.tensor_max(z_g[:, i, :w], zr[:, :w], mt[:, :w])
                else:
                    nc.scalar.activation(z_g[:, i, :w], zps[i][:, :w],
                                         ACT.Sigmoid,
                                         bias=bt[:, bb + 4 + i:bb + 5 + i],
                                         scale=1.0 / SCALE)
                if first:
                    nc.vector.tensor_scalar_mul(tt[:, i, :w], r_g[:, i, :w],
                                                bt[:, bb + 8 + i:bb + 9 + i])
                else:
                    nc.vector.scalar_tensor_tensor(
                        tt[:, i, :w], ghn[i][:, :w],
                        bt[:, bb + 8 + i:bb + 9 + i], r_g[:, i, :w],
                        op0=ALU.add, op1=ALU.mult)
                # ss and the n ACT stay in phase A so the xpn bank and the
                # ss buffer are both freed within the phase (no cross-phase
                # PSUM/buffer WAR edges -> no scheduling deadlock)
                ss = gpool.tile([128, 512], F32, tag="ss", bufs=4, name="ss")
                nc.vector.tensor_add(ss[:, :w], tt[:, i, :w], xpn[i][:, :w])
                nc.scalar.activation(n_g[:, i, :w], ss[:, :w], ACT.Tanh,
                                     bias=bt[:, bb + 12 + i:bb + 13 + i],
                                     scale=1.0 / SCALE)
            return w, so

        def phase_b(d, t, steps, j, st, pa):
            """h update, fp8 cast."""
            w, so = pa
            z_g, n_g = st.z_g, st.n_g
            nsteps = len(steps)
            first = j == 0
            last = j == nsteps - 1
            h_next = hpool.tile([128, 4, S], BF16, tag="hf" if last else "h",
                                bufs=4 if last else 6, name="h")
            if not last:
                nw = steps[j + 1][0]
                nso = S - nw
                if nso < so:
                    nc.gpsimd.memset(h_next[:, :, nso:so], 0.0)
            e = gpool.tile([128, 4, S], BF16, tag="e", bufs=2, name="e")
            dd = None
            if not first:
                dd = gpool.tile([128, 4, S], BF16, tag="dd", bufs=2, name="dd")
            for hh in (0, 2):
                ns = n_g[:, hh:hh + 2, :w]
                zs = z_g[:, hh:hh + 2, :w]
                es = e[:, hh:hh + 2, :w]
                ho = h_next[:, hh:hh + 2, so:]
                if first:
                    nc.vector.tensor_mul(es, zs, ns)
                    nc.vector.tensor_sub(ho, ns, es)
                else:
                    ds = dd[:, hh:hh + 2, :w]
                    nc.vector.tensor_sub(ds, st.h[:, hh:hh + 2, so:], ns)
                    nc.vector.tensor_mul(es, zs, ds)
                    nc.vector.tensor_add(ho, ns, es)
                if not last:
                    h8n = st.h8_next
                    if hh == 0:
                        nc.scalar.activation(h8n[:, hh:hh + 2, nso:],
                                             h_next[:, hh:hh + 2, nso:],
                                             ACT.Copy, scale=1.0)
                    else:
                        nc.vector.tensor_copy(h8n[:, hh:hh + 2, nso:],
                                              h_next[:, hh:hh + 2, nso:])
            st.h = h_next
            st.so = so
            if not last:
                st.h8 = st.h8_next

        # All 4 recurrence chains (dir x tile) are independent: interleave
        # them step-by-step, aligned at their final step, so the PE always
        # has the other chains' matmul streams to hide one chain's gate
        # chain (ACT -> DVE -> cast) latency.  z_g/n_g are persistent
        # per-chain tiles (written in phase A, read in phase B) so pool
        # rotation cannot create forward WAR edges across chains; all other
        # gate tiles are consumed within their own phase.
        chains = [(d, t, sched[t][0 if d == 'f' else 1], St())
                  for d in 'fb' for t in range(ntiles)]
        for ci, (d, t, steps, st) in enumerate(chains):
            st.z_g = const.tile([128, 4, S], BF16, name=f"zg{ci}")
            st.n_g = const.tile([128, 4, S], BF16, name=f"ng{ci}")
        mx = max(len(c[2]) for c in chains)
        for k in range(mx, 0, -1):
            # final round: tile-major order so tile 0's chains (and with
            # them its MLP inputs) complete while tile 1's matmuls stream
            order = [0, 2, 1, 3] if k == 1 else range(len(chains))
            pas = {}
            for ci in order:
                d, t, steps, st = chains[ci]
                if len(steps) >= k:
                    if k > 1:  # next step exists: pre-alloc its h8 tile
                        st.h8_next = h8pool.tile([128, 4, S], F8,
                                                 tag="h8", name="h8")
                    pas[ci] = phase_a(d, t, steps, len(steps) - k, st)
            for ci in order:
                d, t, steps, st = chains[ci]
                if ci in pas:
                    phase_b(d, t, steps, len(steps) - k, st, pas[ci])

        states_f = [c[3] for c in chains[:ntiles]]
        states_b = [c[3] for c in chains[ntiles:]]

        def emit_mlp(t, hf, hb):
            hid = gpool.tile([128, 4, S], BF16, tag="e", bufs=2, name="hid")
            for i in range(4):
                ps = xpps.tile([128, 512], F32, tag="xp", name="mps")
                for k in range(8):
                    src = hf if k < 4 else hb
                    nc.tensor.matmul(ps[:], w1[:, k, i * 128:(i + 1) * 128],
                                     src[:, k % 4, :], start=k == 0, stop=k == 7)
                nc.scalar.activation(hid[:, i, :], ps[:], ACT.Relu,
                                     bias=bt[:, 32 + i:33 + i], scale=1.0 / SCALE)
            onats = [opool.tile([128, H], F32, tag="o", name=f"onat{g}")
                     for g in range(4)]
            ob = gpool.tile([128, 4, S], BF16, tag="dd", bufs=2, name="ob")
            for i in range(4):
                ps = xpps.tile([128, 512], F32, tag="xp", name="ops")
                for k in range(4):
                    nc.tensor.matmul(ps[:], w2[:, k, i * 128:(i + 1) * 128],
                                     hid[:, k, :], start=k == 0, stop=k == 3)
                nc.scalar.activation(ob[:, i, :], ps[:], ACT.Identity,
                                     bias=bt[:, 36 + i:37 + i], scale=1.0 / SCALE)
                for g in range(4):
                    tp = ghps.tile([128, 128], BF16, tag="gh", name="tp")
                    nc.tensor.transpose(tp[:], ob[:, i, g * 128:(g + 1) * 128],
                                        ident[:])
                    nc.vector.tensor_copy(onats[g][:, i * 128:(i + 1) * 128],
                                          tp[:])
            for g in range(4):
                r0 = t * S + g * 128
                nc.sync.dma_start(y_d[r0:r0 + 128, :], onats[g][:])

        for t in range(ntiles):
            emit_mlp(t, states_f[t].h, states_b[t].h)

    nc.compile()
    return nc


def kernel(padded_window, window_len, Wih_f, Whh_f, bih_f, bhh_f,
           Wih_b, Whh_b, bih_b, bhh_b, W1, b1, W2, b2):
    wl = np.asarray(window_len)
    lf = (wl - 1) // 2 + 1
    lb = wl // 2 + 1
    order = np.argsort(wl, kind="stable")

    Bc = B // NCORES
    ntiles = Bc // S
    lf_pc = lf[order].reshape(-1, NCORES)
    lb_pc = lb[order].reshape(-1, NCORES)

    def dir_steps(lens_pc, t):
        seg = lens_pc[t * S:(t + 1) * S]  # [S, NCORES]
        n = int(seg.max())
        steps = []
        for j in range(n):
            need = n - j
            cnt = (seg >= need).sum(axis=0)
            w = int(min(S, max(32, -(-int(cnt.max()) // 32) * 32)))
            masked = bool(cnt.min() < w)
            steps.append((w, masked))
        return tuple(steps)

    sched = tuple((dir_steps(lf_pc, t), dir_steps(lb_pc, t))
                  for t in range(ntiles))

    if sched not in _PROGRAM_CACHE:
        _PROGRAM_CACHE[sched] = _build_program(sched)
    nc = _PROGRAM_CACHE[sched]

    f32, bf16, f8 = np.float32, ml_dtypes.bfloat16, ml_dtypes.float8_e4m3
    sc = np.float32(SCALE)

    def q8(a):
        return np.asarray(a, f32).astype(f8)

    wnf = np.ascontiguousarray(sc * Wih_f[2 * H:].T).astype(bf16)
    wnb = np.ascontiguousarray(sc * Wih_b[2 * H:].T).astype(bf16)
    wrz8f = q8(sc * np.ascontiguousarray(Wih_f[:2 * H].T))
    wrz8b = q8(sc * np.ascontiguousarray(Wih_b[:2 * H].T))
    wh8f = q8(sc * np.ascontiguousarray(Whh_f.T))
    wh8b = q8(sc * np.ascontiguousarray(Whh_b.T))
    w1 = (sc * np.ascontiguousarray(W1.T)).astype(bf16)
    w2 = (sc * np.ascontiguousarray(W2.T)).astype(bf16)

    def chunks(v):  # [512] -> [4, 128]
        return np.asarray(v, f32).reshape(4, 128)

    bias = np.concatenate([
        chunks((bih_f + bhh_f)[:H]), chunks((bih_f + bhh_f)[H:2 * H]),
        chunks(sc * bhh_f[2 * H:]), chunks(bih_f[2 * H:]),
        chunks((bih_b + bhh_b)[:H]), chunks((bih_b + bhh_b)[H:2 * H]),
        chunks(sc * bhh_b[2 * H:]), chunks(bih_b[2 * H:]),
        chunks(b1), chunks(b2),
    ], 0)  # [40, 128]

    pw = np.asarray(padded_window, f32)
    in_maps = []
    p8 = np.arange(8)
    for c in range(NCORES):
        idx = order[c::NCORES]
        xT = np.ascontiguousarray(pw[idx].transpose(1, 2, 0))  # [15, 512, Bc]
        mzf = (p8[:, None] < (8 - lf[idx])[None, :]).astype(bf16)
        mzb = (p8[:, None] < (8 - lb[idx])[None, :]).astype(bf16)
        in_maps.append({
            "xb": xT.astype(bf16), "x8": xT.astype(f8),
            "wnf": wnf, "wnb": wnb, "wrz8f": wrz8f, "wrz8b": wrz8b,
            "wh8f": wh8f, "wh8b": wh8b, "w1": w1, "w2": w2,
            "bias": bias, "maskzf": mzf, "maskzb": mzb,
        })

    trace = bool(os.environ.get("GRU_TRACE"))
    kw = {}
    if os.environ.get("GRU_TMPDIR"):
        kw["tmpdir"] = os.environ["GRU_TMPDIR"]
    res = run_bass_kernel_spmd(nc, in_maps, core_ids=list(range(NCORES)),
                               trace=trace, **kw)
    global LAST_RESULT
    LAST_RESULT = res
    out = np.empty((B, H), f32)
    for c in range(NCORES):
        out[order[c::NCORES]] = res.results[c]["y"]
    return out
